# revision 14
# baseline (speedup 1.0000x reference)
"""Trainium2 Bass kernel for nn_MicroCommunity (scatter_memory).

Strategy: class-sharded across 8 NeuronCores.
  - Classes 0..9999 are split into 8 contiguous shards of 1250 classes.
  - Host sorts batch rows by label and routes each row to the core owning
    its class; within a core, rows are packed into 12 fixed class-bands
    (104/105 classes each), each padded to R=256 rows (max real occupancy
    for the fixed seed is ~209).
  - Each core computes the LSM weights (relu/sigmoid/cumsum/exp chain) for
    its rows, then per band builds a one-hot "T2" matrix
    T2[(c,m), b] = (label_b == c) * norm_w[b, m] and uses PE matmuls:
       memory_band   = T2 @ [data | mask]    (scatter-add + memory_weights)
       center_matrix = T2.T @ centers_band   (gather)
    Loss partials are reduced on-device; host sums 8 scalars.
  - The memory/memory_weights outputs are disjoint across cores (no
    all-reduce needed); host concatenates shards.

kernel() accepts FULL unsharded inputs and returns the FULL outputs
(loss, sum_v, new_memory, new_memory_w) exactly like the reference.
"""

import numpy as np

# ---------------- problem constants (hardcoded per contract) ----------------
BS = 16384
C = 10000
M = 6
D = 256
EPS = 1e-4
NCORES = 8
CPC = C // NCORES          # classes per core = 1250
NB = 12                    # class-bands per core
R = 256                    # padded rows per band
SLOTS = NB * R             # 3072 row slots per core
NRC = SLOTS // 128         # 24 row-chunks of 128 partitions
DENOM_C = EPS + 1e-10      # norm_w denominator epsilon


def _band_class_sizes():
    base = CPC // NB
    rem = CPC - base * NB
    return [base + (1 if b < rem else 0) for b in range(NB)]  # [105,105,104*10]


def _chunk_sizes(nq):
    """Split nq memory rows (multiple of 6) into chunks of <=126 rows,
    each a multiple of 6 (so chunks are class-aligned)."""
    out = []
    while nq > 0:
        c = min(126, nq)
        out.append(c)
        nq -= c
    return out


# ---------------------------- program builder -------------------------------

def build_program():
    import concourse.bass as bass
    import concourse.bacc as bacc
    import concourse.mybir as mybir
    import concourse.tile as tile

    f32 = mybir.dt.float32
    f32r = mybir.dt.float32r
    Alu = mybir.AluOpType
    Act = mybir.ActivationFunctionType

    nc = bacc.Bacc("TRN2", target_bir_lowering=False)

    # ------------- I/O -------------
    data_pad = nc.dram_tensor("data_pad", [SLOTS, D + 2], f32r, kind="ExternalInput")
    w1g = nc.dram_tensor("w1g", [SLOTS, M], f32, kind="ExternalInput")
    beta_in = nc.dram_tensor("beta_in", [SLOTS], f32, kind="ExternalInput")
    mask_in = nc.dram_tensor("mask_in", [SLOTS], f32, kind="ExternalInput")
    lci_in = nc.dram_tensor("lci_in", [SLOTS], f32, kind="ExternalInput")
    cls_row = nc.dram_tensor("cls_row", [CPC], f32, kind="ExternalInput")
    consts_in = nc.dram_tensor("consts_in", [64], f32, kind="ExternalInput")
    centers_sh = nc.dram_tensor("centers_sh", [CPC * M, D], f32r, kind="ExternalInput")
    ident_in = nc.dram_tensor("ident_in", [128, 128], f32r, kind="ExternalInput")

    mem_out = nc.dram_tensor("mem_out", [CPC * M, D], f32, kind="ExternalOutput")
    memw_out = nc.dram_tensor("memw_out", [CPC * M], f32, kind="ExternalOutput")
    sumv_out = nc.dram_tensor("sumv_out", [SLOTS, M], f32, kind="ExternalOutput")
    loss_out = nc.dram_tensor("loss_out", [1, 1], f32, kind="ExternalOutput")

    band_sizes = _band_class_sizes()
    band_c0 = np.cumsum([0] + band_sizes).tolist()  # class offset of each band

    def apx(ap, dims, extra=0):
        return bass.AP(tensor=ap.tensor, offset=ap.offset + extra, ap=dims)

    with tile.TileContext(nc) as tc:
        with (
            tc.tile_pool(name="singles", bufs=1) as singles,
            tc.tile_pool(name="dpool", bufs=4) as dpool,
            tc.tile_pool(name="cpool", bufs=8) as cpool,
            tc.tile_pool(name="t2pool", bufs=8) as t2pool,
            tc.tile_pool(name="lspool", bufs=3) as lspool,
            tc.tile_pool(name="ohpool", bufs=3) as ohpool,
            tc.tile_pool(name="dfpool", bufs=3) as dfpool,
            tc.tile_pool(name="trpool", bufs=2) as trpool,
            tc.tile_pool(name="stpool", bufs=4) as stpool,
            tc.tile_pool(name="pT", bufs=2, space="PSUM") as pT,
            tc.tile_pool(name="pSC", bufs=3, space="PSUM") as pSC,
            tc.tile_pool(name="pCM", bufs=2, space="PSUM") as pCM,
            tc.tile_pool(name="pL", bufs=1, space="PSUM") as pL,
        ):
            # ---------------- resident tiles + loads ----------------
            ident = singles.tile([128, 128], f32r, tag="ident")
            nc.sync.dma_start(out=ident[:], in_=ident_in[:])

            clsrep = singles.tile([128, CPC], f32, tag="clsrep")
            nc.sync.dma_start(
                out=clsrep[:], in_=apx(cls_row[:], [[0, 128], [1, CPC]])
            )
            consts = singles.tile([128, 64], f32, tag="consts")
            nc.sync.dma_start(
                out=consts[:], in_=apx(consts_in[:], [[0, 128], [1, 64]])
            )

            w1g_t = singles.tile([128, NRC, M], f32, tag="w1g")
            nc.sync.dma_start(
                out=w1g_t[:], in_=apx(w1g[:], [[M, 128], [128 * M, NRC], [1, M]])
            )
            beta_t = singles.tile([128, NRC], f32, tag="beta")
            nc.sync.dma_start(
                out=beta_t[:], in_=apx(beta_in[:], [[1, 128], [128, NRC]])
            )
            mask_t = singles.tile([128, NRC], f32, tag="mask")
            nc.sync.dma_start(
                out=mask_t[:], in_=apx(mask_in[:], [[1, 128], [128, NRC]])
            )
            lci_t = singles.tile([128, NRC], f32, tag="lci")
            nc.sync.dma_start(
                out=lci_t[:], in_=apx(lci_in[:], [[1, 128], [128, NRC]])
            )

            # ---------------- phase A: norm_w / sum_v ----------------
            F = NRC * M  # 144 free elems
            h_t = singles.tile([128, NRC, M], f32, tag="h")
            z_t = singles.tile([128, NRC, M], f32, tag="z")
            tmp_t = singles.tile([128, NRC, M], f32, tag="tmpa")
            s_t = singles.tile([128, NRC, M], f32, tag="s")
            cs_t = singles.tile([128, NRC, M], f32, tag="cs")
            w_t = singles.tile([128, NRC, M], f32, tag="w")
            nw_t = singles.tile([128, NRC, M], f32, tag="nw")
            sw_t = singles.tile([128, NRC], f32, tag="sw")
            losspart = singles.tile([128, NRC], f32, tag="losspart")

            cst = consts[:]  # [128, 64]; strides [[64,128],[1,64]]
            PSTRIDE = 64

            c_sqrt_bias = singles.tile([128, 1], f32, tag="csqrtb")
            nc.vector.memset(c_sqrt_bias[:], 1e-10)

            def cbc(off, dims):
                return apx(cst, dims, extra=off)

            # h = relu(w1g + b1)   (b1 at consts[36:42])
            nc.vector.tensor_tensor(
                out=h_t[:], in0=w1g_t[:],
                in1=cbc(36, [[PSTRIDE, 128], [0, NRC], [1, M]]),
                op=Alu.add,
            )
            nc.vector.tensor_scalar_max(out=h_t[:], in0=h_t[:], scalar1=0.0)

            # z = h @ W2.T  (W2 row-major at consts[0:36]; W2[j,k] at 6j+k)
            ht = h_t[:]
            HS = NRC * M  # partition stride of h tile
            for k in range(M):
                dst = z_t[:] if k == 0 else tmp_t[:]
                nc.vector.tensor_tensor(
                    out=dst,
                    in0=apx(ht, [[HS, 128], [M, NRC], [0, M]], extra=k),
                    in1=cbc(k, [[PSTRIDE, 128], [0, NRC], [M, M]]),
                    op=Alu.mult,
                )
                if k > 0:
                    nc.vector.tensor_tensor(
                        out=z_t[:], in0=z_t[:], in1=tmp_t[:], op=Alu.add
                    )
            # z += b2  (consts[42:48])
            nc.vector.tensor_tensor(
                out=z_t[:], in0=z_t[:],
                in1=cbc(42, [[PSTRIDE, 128], [0, NRC], [1, M]]),
                op=Alu.add,
            )
            # s = sigmoid(z) + EPS
            nc.scalar.activation(out=s_t[:], in_=z_t[:], func=Act.Sigmoid)
            nc.vector.tensor_scalar_add(out=s_t[:], in0=s_t[:], scalar1=EPS)
            # cs = cumsum(s, axis=-1)
            nc.vector.tensor_copy(out=cs_t[:, :, 0], in_=s_t[:, :, 0])
            for j in range(1, M):
                nc.vector.tensor_tensor(
                    out=cs_t[:, :, j], in0=cs_t[:, :, j - 1], in1=s_t[:, :, j],
                    op=Alu.add,
                )
            # sum_v output
            nc.sync.dma_start(
                out=apx(sumv_out[:], [[M, 128], [128 * M, NRC], [1, M]]),
                in_=cs_t[:],
            )
            # w = exp(-sqrt((beta-cs)^2 + 1e-10))
            bt = beta_t[:]
            nc.vector.tensor_tensor(
                out=w_t[:],
                in0=apx(bt, [[NRC, 128], [1, NRC], [0, M]]),
                in1=cs_t[:], op=Alu.subtract,
            )
            nc.vector.tensor_tensor(out=w_t[:], in0=w_t[:], in1=w_t[:], op=Alu.mult)
            nc.scalar.activation(
                out=w_t[:], in_=w_t[:], func=Act.Sqrt, bias=c_sqrt_bias[:]
            )
            nc.scalar.activation(out=w_t[:], in_=w_t[:], func=Act.Exp, scale=-1.0)
            # sw = sum(w) + DENOM_C ; nw = (w / sw) * mask
            nc.vector.tensor_reduce(
                out=sw_t[:], in_=w_t[:], axis=mybir.AxisListType.X, op=Alu.add
            )
            nc.vector.tensor_scalar_add(out=sw_t[:], in0=sw_t[:], scalar1=DENOM_C)
            # fold mask into w first (padded rows -> nw = 0)
            nc.vector.tensor_tensor(
                out=w_t[:], in0=w_t[:],
                in1=apx(mask_t[:], [[NRC, 128], [1, NRC], [0, M]]),
                op=Alu.mult,
            )
            rw_t = singles.tile([128, NRC], f32, tag="rw")
            nc.vector.reciprocal(out=rw_t[:], in_=sw_t[:])
            nc.vector.tensor_tensor(
                out=nw_t[:], in0=w_t[:],
                in1=apx(rw_t[:], [[NRC, 128], [1, NRC], [0, M]]),
                op=Alu.mult,
            )

            # ---------------- phase B: per-band gather/scatter ----------------
            for b in range(NB):
                B = band_sizes[b]
                c0 = band_c0[b]
                nq = 6 * B
                chunks = _chunk_sizes(nq)
                qoffs = np.cumsum([0] + chunks).tolist()
                NJ = len(chunks)

                # loads
                dts = []
                for rc in range(2):
                    r0 = b * R + rc * 128
                    dt = dpool.tile([128, D + 2], f32r, tag="dt")
                    nc.sync.dma_start(out=dt[:], in_=data_pad[r0:r0 + 128, :])
                    dts.append(dt)
                cts = []
                for j in range(NJ):
                    ct = cpool.tile([126, D], f32r, tag="ct")
                    rbase = c0 * M + qoffs[j]
                    nc.sync.dma_start(
                        out=ct[:chunks[j], :],
                        in_=centers_sh[rbase:rbase + chunks[j], :],
                    )
                    cts.append(ct)

                # build lhsT_sc [128, nq] per row-chunk:
                #   lhsT_sc[p, (c',m)] = (lci==c0+c') * nw[p, m]
                lhsts = []
                for rc in range(2):
                    g = b * 2 + rc
                    oht = ohpool.tile([128, 105], f32, tag="oht")
                    nc.vector.tensor_scalar(
                        out=oht[:, :B],
                        in0=clsrep[:, c0:c0 + B],
                        scalar1=lci_t[:, g:g + 1],
                        scalar2=None,
                        op0=Alu.is_equal,
                    )
                    lh = lspool.tile([128, 6 * 105], f32r, tag="lh")
                    ohap = oht[:]
                    nwap = nw_t[:]
                    nc.gpsimd.tensor_tensor(
                        out=lh[:, :nq],
                        in0=apx(ohap, [[105, 128], [1, B], [0, M]]),
                        in1=apx(nwap, [[NRC * M, 128], [0, B], [1, M]], extra=g * M),
                        op=Alu.mult,
                    )
                    lhsts.append(lh)

                # transposes -> T2 chunks [csz, 256]
                t2s = []
                for j in range(NJ):
                    csz = chunks[j]
                    pt = pT.tile([126, 256], f32, tag="pT")
                    for rc in range(2):
                        nc.tensor.transpose(
                            out=pt[:csz, rc * 128:(rc + 1) * 128].bitcast(f32r),
                            in_=lhsts[rc][:, qoffs[j]:qoffs[j] + csz],
                            identity=ident[:],
                        )
                    t2 = t2pool.tile([126, 256], f32r, tag="t2")
                    if j % 2 == 0:
                        nc.vector.tensor_copy(out=t2[:csz, :], in_=pt[:csz, :])
                    else:
                        nc.scalar.activation(
                            out=t2[:csz, :], in_=pt[:csz, :], func=Act.Copy
                        )
                    t2s.append(t2)

                # scatter matmuls + output DMA
                for j in range(NJ):
                    csz = chunks[j]
                    psc = pSC.tile([126, D + 2], f32, tag="pSC")
                    for rc in range(2):
                        nc.tensor.matmul(
                            out=psc[:csz, :],
                            lhsT=lhsts[rc][:, qoffs[j]:qoffs[j] + csz],
                            rhs=dts[rc][:],
                            start=(rc == 0),
                            stop=(rc == 1),
                        )
                    stg = stpool.tile([126, D + 2], f32, tag="stg")
                    if j % 2 == 0:
                        nc.vector.tensor_copy(out=stg[:csz, :], in_=psc[:csz, :])
                    else:
                        nc.scalar.activation(
                            out=stg[:csz, :], in_=psc[:csz, :], func=Act.Copy
                        )
                    rbase = c0 * M + qoffs[j]
                    nc.sync.dma_start(
                        out=mem_out[rbase:rbase + csz, :], in_=stg[:csz, 0:D]
                    )
                    nc.sync.dma_start(
                        out=memw_out[rbase:rbase + csz], in_=stg[:csz, D:D + 1]
                    )

                # gather matmuls + loss
                for rc in range(2):
                    g = b * 2 + rc
                    pcm = pCM.tile([128, D], f32, tag="pCM")
                    for j in range(NJ):
                        csz = chunks[j]
                        nc.tensor.matmul(
                            out=pcm[:],
                            lhsT=t2s[j][:csz, rc * 128:(rc + 1) * 128],
                            rhs=cts[j][:csz, :],
                            start=(j == 0),
                            stop=(j == NJ - 1),
                        )
                    diff = dfpool.tile([128, D], f32, tag="diff")
                    nc.vector.tensor_tensor(
                        out=diff[:], in0=dts[rc][:, 0:D].bitcast(f32), in1=pcm[:], op=Alu.subtract
                    )
                    trash = trpool.tile([128, D], f32, tag="trash")
                    nc.scalar.activation(
                        out=trash[:], in_=diff[:], func=Act.Square,
                        accum_out=losspart[:, g:g + 1],
                    )

            # ---------------- final loss reduction ----------------
            lp1 = singles.tile([128, 1], f32, tag="lp1")
            nc.vector.tensor_reduce(
                out=lp1[:], in_=losspart[:], axis=mybir.AxisListType.X, op=Alu.add
            )
            ones_t = singles.tile([128, 1], f32, tag="ones")
            nc.vector.memset(ones_t[:], 1.0)
            pl = pL.tile([1, 1], f32, tag="pL")
            nc.tensor.matmul(
                out=pl[:], lhsT=lp1[:], rhs=ones_t[:],
                start=True, stop=True,
            )
            lt = singles.tile([1, 1], f32, tag="lt")
            nc.vector.tensor_copy(out=lt[:], in_=pl[:])
            nc.sync.dma_start(out=loss_out[:], in_=lt[:])

    nc.compile()
    return nc


# ------------------------------ host side -----------------------------------

def _prep_inputs(data, source_labels, beta, W1):
    """Sort rows by label, pack into per-core band-padded layouts."""
    labels = np.asarray(source_labels)
    data = np.asarray(data, dtype=np.float32)
    beta = np.asarray(beta, dtype=np.float32)
    W1T = np.asarray(W1, dtype=np.float32).T  # [C, M]

    order = np.argsort(labels, kind="stable")
    slab = labels[order]

    band_sizes = _band_class_sizes()
    band_c0 = np.cumsum([0] + band_sizes)

    in_maps = []
    perms = []
    for k in range(NCORES):
        dp = np.zeros((SLOTS, D + 2), np.float32)
        w1g = np.zeros((SLOTS, M), np.float32)
        betam = np.zeros(SLOTS, np.float32)
        maskm = np.zeros(SLOTS, np.float32)
        lci = np.zeros(SLOTS, np.float32)
        perm = np.full(SLOTS, -1, np.int64)
        for b in range(NB):
            glo = k * CPC + band_c0[b]
            ghi = k * CPC + band_c0[b + 1]
            lo = np.searchsorted(slab, glo, side="left")
            hi = np.searchsorted(slab, ghi, side="left")
            n = hi - lo
            if n > R:
                raise OverflowError("band overflow")
            rows = order[lo:hi]
            s0 = b * R
            dp[s0:s0 + n, :D] = data[rows]
            dp[s0:s0 + n, D] = 1.0
            w1g[s0:s0 + n] = W1T[labels[rows]]
            betam[s0:s0 + n] = beta[rows]
            maskm[s0:s0 + n] = 1.0
            lci[s0:s0 + n] = (labels[rows] - k * CPC).astype(np.float32)
            lci[s0 + n:s0 + R] = float(band_c0[b])
            perm[s0:s0 + n] = rows
        in_maps.append({
            "data_pad": dp,
            "w1g": w1g,
            "beta_in": betam,
            "mask_in": maskm,
            "lci_in": lci,
        })
        perms.append(perm)
    return in_maps, perms


_PROGRAM_CACHE = {}


def kernel(data, source_labels, beta, centers, W1, b1, W2, b2,
           memory, memory_weights):
    data = np.asarray(data)
    source_labels = np.asarray(source_labels)
    beta = np.asarray(beta)
    centers = np.asarray(centers, dtype=np.float32)
    W1 = np.asarray(W1, dtype=np.float32)
    b1 = np.asarray(b1, dtype=np.float32)
    W2 = np.asarray(W2, dtype=np.float32)
    b2 = np.asarray(b2, dtype=np.float32)
    memory = np.asarray(memory, dtype=np.float32)
    memory_weights = np.asarray(memory_weights, dtype=np.float32)

    try:
        in_maps, perms = _prep_inputs(data, source_labels, beta, W1)
    except OverflowError:
        return _numpy_fallback(data, source_labels, beta, centers, W1, b1,
                               W2, b2, memory, memory_weights)

    consts = np.zeros(64, np.float32)
    consts[0:36] = W2.reshape(-1)
    consts[36:42] = b1
    consts[42:48] = b2
    cls_row = np.arange(CPC, dtype=np.float32)

    ident = np.eye(128, dtype=np.float32)
    for k in range(NCORES):
        in_maps[k]["ident_in"] = ident
        in_maps[k]["cls_row"] = cls_row
        in_maps[k]["consts_in"] = consts
        in_maps[k]["centers_sh"] = np.ascontiguousarray(
            centers[k * CPC * M:(k + 1) * CPC * M]
        )

    if "nc" not in _PROGRAM_CACHE:
        _PROGRAM_CACHE["nc"] = build_program()
    nc = _PROGRAM_CACHE["nc"]

    from concourse.bass_utils import run_bass_kernel_spmd
    res = run_bass_kernel_spmd(nc, in_maps, list(range(NCORES)))
    results = res.results

    # assemble full outputs
    loss_sum = np.float32(0.0)
    sum_v = np.zeros((BS, M), np.float32)
    mem = np.empty((C * M, D), np.float32)
    memw = np.empty(C * M, np.float32)
    for k in range(NCORES):
        r = results[k]
        loss_sum += r["loss_out"].reshape(-1)[0]
        sv = r["sumv_out"].reshape(SLOTS, M)
        valid = perms[k] >= 0
        sum_v[perms[k][valid]] = sv[valid]
        mem[k * CPC * M:(k + 1) * CPC * M] = r["mem_out"].reshape(CPC * M, D)
        memw[k * CPC * M:(k + 1) * CPC * M] = r["memw_out"].reshape(-1)

    loss = np.float32(loss_sum / (BS * D))
    new_memory = mem
    new_memory_w = memw.reshape(C * M, 1)
    if memory.any():
        new_memory = new_memory + memory.reshape(C * M, D)
    if memory_weights.any():
        new_memory_w = new_memory_w + memory_weights.reshape(C * M, 1)
    return loss, sum_v, new_memory, new_memory_w


# ---------------------- numpy fallback (safety net) --------------------------

def _numpy_fallback(data, source_labels, beta, centers, W1, b1, W2, b2,
                    memory, memory_weights):
    labels = np.asarray(source_labels)
    h = np.maximum(W1.T[labels] + b1, 0.0)
    out = 1.0 / (1.0 + np.exp(-(h @ W2.T + b2))) + EPS
    cs = np.cumsum(out, axis=1)
    val = (beta[:, None] - cs) ** 2
    w = np.exp(-np.sqrt(val + 1e-10))
    nw = w / (w.sum(axis=1, keepdims=True) + EPS + 1e-10)
    centers3 = centers.reshape(C, M, D)
    cm = np.einsum("bmd,bm->bd", centers3[labels], nw)
    loss = np.float32(np.mean((data - cm) ** 2))
    feat = data[:, None, :] * nw[:, :, None]
    new_mem = memory.reshape(C, M, D).copy()
    np.add.at(new_mem, labels, feat)
    new_mw = memory_weights.reshape(C, M).copy()
    np.add.at(new_mw, labels, nw)
    return (loss, cs.astype(np.float32),
            new_mem.reshape(C * M, D).astype(np.float32),
            new_mw.reshape(C * M, 1).astype(np.float32))


# revision 18
# speedup vs baseline: 2.2822x; 2.2822x over previous
"""Trainium2 Bass kernel for nn_MicroCommunity (scatter_memory).

Strategy: class-sharded across 8 NeuronCores.
  - Classes 0..9999 are split into 8 contiguous shards of 1250 classes.
  - Host sorts batch rows by label and routes each row to the core owning
    its class; within a core, rows are packed into 12 fixed class-bands
    (104/105 classes each), each padded to R=256 rows (max real occupancy
    for the fixed seed is ~209).
  - Each core computes the LSM weights (relu/sigmoid/cumsum/exp chain) for
    its rows, then per band builds a one-hot "T2" matrix
    T2[(c,m), b] = (label_b == c) * norm_w[b, m] and uses PE matmuls:
       memory_band   = T2 @ [data | mask]    (scatter-add + memory_weights)
       center_matrix = T2.T @ centers_band   (gather)
    Loss partials are reduced on-device; host sums 8 scalars.
  - The memory/memory_weights outputs are disjoint across cores (no
    all-reduce needed); host concatenates shards.

kernel() accepts FULL unsharded inputs and returns the FULL outputs
(loss, sum_v, new_memory, new_memory_w) exactly like the reference.
"""

import numpy as np

# ---------------- problem constants (hardcoded per contract) ----------------
BS = 16384
C = 10000
M = 6
D = 256
EPS = 1e-4
NCORES = 8
CPC = C // NCORES          # classes per core = 1250
NB = 12                    # class-bands per core
R = 256                    # padded rows per band
SLOTS = NB * R             # 3072 row slots per core
NRC = SLOTS // 128         # 24 row-chunks of 128 partitions
DENOM_C = EPS + 1e-10      # norm_w denominator epsilon
SMALLW = NRC * M + 3 * NRC  # packed small input width: 144 + 72 = 216
NCHUNK = 60                 # total (band, chunk) pairs per core


def _band_class_sizes():
    base = CPC // NB
    rem = CPC - base * NB
    return [base + (1 if b < rem else 0) for b in range(NB)]  # [105,105,104*10]


def _chunk_sizes(nq):
    out = []
    while nq > 0:
        c = min(126, nq)
        out.append(c)
        nq -= c
    return out


# ---------------------------- program builder -------------------------------

def build_program():
    import concourse.bass as bass
    import concourse.bacc as bacc
    import concourse.mybir as mybir
    import concourse.tile as tile

    f32 = mybir.dt.float32
    f32r = mybir.dt.float32r
    Alu = mybir.AluOpType
    Act = mybir.ActivationFunctionType

    nc = bacc.Bacc("TRN2", target_bir_lowering=False)

    # ------------- I/O -------------
    data_pad = nc.dram_tensor("data_pad", [SLOTS, D + 2], f32r, kind="ExternalInput")
    small_in = nc.dram_tensor("small_in", [128, SMALLW], f32, kind="ExternalInput")
    cls_row = nc.dram_tensor("cls_row", [CPC], f32, kind="ExternalInput")
    consts_in = nc.dram_tensor("consts_in", [64], f32, kind="ExternalInput")
    centers_sh = nc.dram_tensor("centers_sh", [CPC * M, D], f32r, kind="ExternalInput")
    ident_in = nc.dram_tensor("ident_in", [128, 128], f32r, kind="ExternalInput")

    mem_out = nc.dram_tensor("mem_out", [CPC * M, D], f32, kind="ExternalOutput")
    memw_out = nc.dram_tensor("memw_out", [126, NCHUNK], f32, kind="ExternalOutput")
    sumv_out = nc.dram_tensor("sumv_out", [128, NRC * M], f32, kind="ExternalOutput")
    loss_out = nc.dram_tensor("loss_out", [1, 1], f32, kind="ExternalOutput")

    band_sizes = _band_class_sizes()
    band_c0 = np.cumsum([0] + band_sizes).tolist()

    def apx(ap, dims, extra=0):
        return bass.AP(tensor=ap.tensor, offset=ap.offset + extra, ap=dims)

    with tile.TileContext(nc) as tc:
        with (
            tc.tile_pool(name="singles", bufs=1) as singles,
            tc.tile_pool(name="dpool", bufs=3) as dpool,
            tc.tile_pool(name="cpool", bufs=3) as cpool,
            tc.tile_pool(name="t2pool", bufs=8) as t2pool,
            tc.tile_pool(name="lspool", bufs=3) as lspool,
            tc.tile_pool(name="ohpool", bufs=3) as ohpool,
            tc.tile_pool(name="dfpool", bufs=3) as dfpool,
            tc.tile_pool(name="trpool", bufs=2) as trpool,
            tc.tile_pool(name="stpool", bufs=3) as stpool,
            tc.tile_pool(name="pT", bufs=2, space="PSUM") as pT,
            tc.tile_pool(name="pSC", bufs=3, space="PSUM") as pSC,
            tc.tile_pool(name="pCM", bufs=2, space="PSUM") as pCM,
            tc.tile_pool(name="pL", bufs=1, space="PSUM") as pL,
        ):
            # ---------------- resident tiles + loads ----------------
            ident = singles.tile([128, 128], f32r, tag="ident")
            nc.sync.dma_start(out=ident[:], in_=ident_in[:])

            clsrep = singles.tile([128, CPC], f32, tag="clsrep")
            nc.sync.dma_start(
                out=clsrep[:], in_=apx(cls_row[:], [[0, 128], [1, CPC]])
            )
            consts = singles.tile([128, 64], f32, tag="consts")
            nc.sync.dma_start(
                out=consts[:], in_=apx(consts_in[:], [[0, 128], [1, 64]])
            )

            small_t = singles.tile([128, SMALLW], f32, tag="small")
            nc.sync.dma_start(out=small_t[:], in_=small_in[:])
            st = small_t[:]
            SP = SMALLW  # partition stride of small tile
            F = NRC * M  # 144

            def w1g_v():
                return apx(st, [[SP, 128], [M, NRC], [1, M]], extra=0)

            def beta_v3():
                return apx(st, [[SP, 128], [1, NRC], [0, M]], extra=F)

            def mask_v3():
                return apx(st, [[SP, 128], [1, NRC], [0, M]], extra=F + NRC)

            def lci_col(g):
                return apx(st, [[SP, 128], [1, 1]], extra=F + 2 * NRC + g)

            # ---------------- phase A: norm_w / sum_v ----------------
            h_t = singles.tile([128, NRC, M], f32, tag="h")
            z_t = singles.tile([128, NRC, M], f32, tag="z")
            tmp_t = singles.tile([128, NRC, M], f32, tag="tmpa")
            s_t = singles.tile([128, NRC, M], f32, tag="s")
            cs_t = singles.tile([128, NRC, M], f32, tag="cs")
            w_t = singles.tile([128, NRC, M], f32, tag="w")
            nw_t = singles.tile([128, NRC, M], f32, tag="nw")
            sw_t = singles.tile([128, NRC], f32, tag="sw")
            rw_t = singles.tile([128, NRC], f32, tag="rw")
            losspart = singles.tile([128, NRC], f32, tag="losspart")

            cst = consts[:]
            PSTRIDE = 64

            c_sqrt_bias = singles.tile([128, 1], f32, tag="csqrtb")
            nc.vector.memset(c_sqrt_bias[:], 1e-10)

            def cbc(off, dims):
                return apx(cst, dims, extra=off)

            # h = relu(w1g + b1)   (b1 at consts[36:42])
            nc.vector.tensor_tensor(
                out=h_t[:], in0=w1g_v(),
                in1=cbc(36, [[PSTRIDE, 128], [0, NRC], [1, M]]),
                op=Alu.add,
            )
            nc.vector.tensor_scalar_max(out=h_t[:], in0=h_t[:], scalar1=0.0)

            # z = h @ W2.T  (W2 row-major at consts[0:36])
            ht = h_t[:]
            HS = NRC * M
            for k in range(M):
                dst = z_t[:] if k == 0 else tmp_t[:]
                nc.vector.tensor_tensor(
                    out=dst,
                    in0=apx(ht, [[HS, 128], [M, NRC], [0, M]], extra=k),
                    in1=cbc(k, [[PSTRIDE, 128], [0, NRC], [M, M]]),
                    op=Alu.mult,
                )
                if k > 0:
                    nc.vector.tensor_tensor(
                        out=z_t[:], in0=z_t[:], in1=tmp_t[:], op=Alu.add
                    )
            nc.vector.tensor_tensor(
                out=z_t[:], in0=z_t[:],
                in1=cbc(42, [[PSTRIDE, 128], [0, NRC], [1, M]]),
                op=Alu.add,
            )
            # s = sigmoid(z) + EPS
            nc.scalar.activation(out=s_t[:], in_=z_t[:], func=Act.Sigmoid)
            nc.vector.tensor_scalar_add(out=s_t[:], in0=s_t[:], scalar1=EPS)
            # cs = cumsum(s, axis=-1)
            nc.vector.tensor_copy(out=cs_t[:, :, 0], in_=s_t[:, :, 0])
            for j in range(1, M):
                nc.vector.tensor_tensor(
                    out=cs_t[:, :, j], in0=cs_t[:, :, j - 1], in1=s_t[:, :, j],
                    op=Alu.add,
                )
            # sum_v output (tiled layout, host de-tiles)
            nc.sync.dma_start(out=sumv_out[:], in_=cs_t[:])
            # w = exp(-sqrt((beta-cs)^2 + 1e-10))
            nc.vector.tensor_tensor(
                out=w_t[:], in0=beta_v3(), in1=cs_t[:], op=Alu.subtract
            )
            nc.vector.tensor_tensor(out=w_t[:], in0=w_t[:], in1=w_t[:], op=Alu.mult)
            nc.scalar.activation(
                out=w_t[:], in_=w_t[:], func=Act.Sqrt, bias=c_sqrt_bias[:]
            )
            nc.scalar.activation(out=w_t[:], in_=w_t[:], func=Act.Exp, scale=-1.0)
            # sw = sum(w) + DENOM_C ; nw = (w * recip(sw)) * mask
            nc.vector.tensor_reduce(
                out=sw_t[:], in_=w_t[:], axis=mybir.AxisListType.X, op=Alu.add
            )
            nc.vector.tensor_scalar_add(out=sw_t[:], in0=sw_t[:], scalar1=DENOM_C)
            nc.vector.tensor_tensor(
                out=w_t[:], in0=w_t[:], in1=mask_v3(), op=Alu.mult
            )
            nc.vector.reciprocal(out=rw_t[:], in_=sw_t[:])
            nc.vector.tensor_tensor(
                out=nw_t[:], in0=w_t[:],
                in1=apx(rw_t[:], [[NRC, 128], [1, NRC], [0, M]]),
                op=Alu.mult,
            )

            memw_sb = singles.tile([126, NCHUNK], f32, tag="memw")
            nc.vector.memset(memw_sb[:], 0.0)

            # ---------------- phase B: per-band gather/scatter ----------------
            gchunk = 0
            for b in range(NB):
                B = band_sizes[b]
                c0 = band_c0[b]
                nq = 6 * B
                chunks = _chunk_sizes(nq)
                qoffs = np.cumsum([0] + chunks).tolist()
                NJ = len(chunks)

                # merged loads: data (1 DMA), centers (1 DMA, scalar ring)
                dt2 = dpool.tile([128, 2, D + 2], f32r, tag="dt")
                r0 = b * R
                nc.sync.dma_start(
                    out=dt2[:],
                    in_=apx(data_pad[:], [[D + 2, 128], [128 * (D + 2), 2], [1, D + 2]],
                            extra=r0 * (D + 2)),
                )
                ct2 = cpool.tile([126, NJ, D], f32r, tag="ct")
                rbase0 = c0 * M
                if chunks[-1] == 126:
                    nc.scalar.dma_start(
                        out=ct2[:],
                        in_=apx(centers_sh[:], [[D, 126], [126 * D, NJ], [1, D]],
                                extra=rbase0 * D),
                    )
                else:
                    nc.scalar.dma_start(
                        out=ct2[:, 0:NJ - 1, :],
                        in_=apx(centers_sh[:],
                                [[D, 126], [126 * D, NJ - 1], [1, D]],
                                extra=rbase0 * D),
                    )
                    cl = chunks[-1]
                    nc.scalar.dma_start(
                        out=ct2[:cl, NJ - 1, :],
                        in_=centers_sh[rbase0 + 126 * (NJ - 1):
                                       rbase0 + 126 * (NJ - 1) + cl, :],
                    )

                # build lhsT_sc [128, nq] per row-chunk
                lhsts = []
                for rc in range(2):
                    g = b * 2 + rc
                    oht = ohpool.tile([128, 105], f32, tag="oht")
                    nc.vector.tensor_tensor(
                        out=oht[:, :B],
                        in0=clsrep[:, c0:c0 + B],
                        in1=lci_col(g).to_broadcast([128, B]),
                        op=Alu.is_equal,
                    )
                    lh = lspool.tile([128, 6 * 105], f32r, tag="lh")
                    nc.gpsimd.tensor_tensor(
                        out=lh[:, :nq],
                        in0=apx(oht[:], [[105, 128], [1, B], [0, M]]),
                        in1=apx(nw_t[:], [[NRC * M, 128], [0, B], [1, M]],
                                extra=g * M),
                        op=Alu.mult,
                    )
                    lhsts.append(lh)

                # transposes -> T2 chunks [csz, 256]
                t2s = []
                for j in range(NJ):
                    csz = chunks[j]
                    pt = pT.tile([126, 256], f32, tag="pT")
                    for rc in range(2):
                        nc.tensor.transpose(
                            out=pt[:csz, rc * 128:(rc + 1) * 128].bitcast(f32r),
                            in_=lhsts[rc][:, qoffs[j]:qoffs[j] + csz],
                            identity=ident[:],
                        )
                    t2 = t2pool.tile([126, 256], f32r, tag="t2")
                    nc.scalar.activation(
                        out=t2[:csz, :], in_=pt[:csz, :], func=Act.Copy
                    )
                    t2s.append(t2)

                # scatter matmuls + staging + memw column
                stg2 = stpool.tile([126, NJ, D], f32, tag="stg")
                for j in range(NJ):
                    csz = chunks[j]
                    psc = pSC.tile([126, D + 2], f32, tag="pSC")
                    for rc in range(2):
                        nc.tensor.matmul(
                            out=psc[:csz, :],
                            lhsT=lhsts[rc][:, qoffs[j]:qoffs[j] + csz],
                            rhs=dt2[:, rc, :],
                            start=(rc == 0),
                            stop=(rc == 1),
                        )
                    if j % 2 == 0:
                        nc.vector.tensor_copy(
                            out=stg2[:csz, j, :], in_=psc[:csz, 0:D]
                        )
                    else:
                        nc.scalar.activation(
                            out=stg2[:csz, j, :], in_=psc[:csz, 0:D], func=Act.Copy
                        )
                    gj = gchunk + j
                    nc.vector.tensor_copy(
                        out=memw_sb[:csz, gj:gj + 1], in_=psc[:csz, D:D + 1]
                    )
                if chunks[-1] == 126:
                    nc.sync.dma_start(
                        out=apx(mem_out[:], [[D, 126], [126 * D, NJ], [1, D]],
                                extra=rbase0 * D),
                        in_=stg2[:],
                    )
                else:
                    nc.sync.dma_start(
                        out=apx(mem_out[:], [[D, 126], [126 * D, NJ - 1], [1, D]],
                                extra=rbase0 * D),
                        in_=stg2[:, 0:NJ - 1, :],
                    )
                    cl = chunks[-1]
                    nc.sync.dma_start(
                        out=mem_out[rbase0 + qoffs[NJ - 1]:
                                    rbase0 + qoffs[NJ - 1] + cl, :],
                        in_=stg2[:cl, NJ - 1, :],
                    )

                # gather matmuls + loss
                for rc in range(2):
                    g = b * 2 + rc
                    pcm = pCM.tile([128, D], f32, tag="pCM")
                    for j in range(NJ):
                        csz = chunks[j]
                        nc.tensor.matmul(
                            out=pcm[:],
                            lhsT=t2s[j][:csz, rc * 128:(rc + 1) * 128],
                            rhs=ct2[:csz, j, :],
                            start=(j == 0),
                            stop=(j == NJ - 1),
                        )
                    diff = dfpool.tile([128, D], f32, tag="diff")
                    nc.vector.tensor_tensor(
                        out=diff[:], in0=dt2[:, rc, 0:D].bitcast(f32),
                        in1=pcm[:], op=Alu.subtract,
                    )
                    trash = trpool.tile([128, D], f32, tag="trash")
                    nc.scalar.activation(
                        out=trash[:], in_=diff[:], func=Act.Square,
                        accum_out=losspart[:, g:g + 1],
                    )
                gchunk += NJ

            # memory_weights output (tiled; host de-tiles)
            nc.sync.dma_start(out=memw_out[:], in_=memw_sb[:])

            # ---------------- final loss reduction ----------------
            lp1 = singles.tile([128, 1], f32, tag="lp1")
            nc.vector.tensor_reduce(
                out=lp1[:], in_=losspart[:], axis=mybir.AxisListType.X, op=Alu.add
            )
            ones_t = singles.tile([128, 1], f32, tag="ones")
            nc.vector.memset(ones_t[:], 1.0)
            pl = pL.tile([1, 1], f32, tag="pL")
            nc.tensor.matmul(
                out=pl[:], lhsT=lp1[:], rhs=ones_t[:], start=True, stop=True
            )
            lt = singles.tile([1, 1], f32, tag="lt")
            nc.vector.tensor_copy(out=lt[:], in_=pl[:])
            nc.sync.dma_start(out=loss_out[:], in_=lt[:])

    nc.compile()
    return nc


# ------------------------------ host side -----------------------------------

def _prep_inputs(data, source_labels, beta, W1):
    """Sort rows by label, pack into per-core band-padded layouts."""
    labels = np.asarray(source_labels)
    data = np.asarray(data, dtype=np.float32)
    beta = np.asarray(beta, dtype=np.float32)
    W1T = np.asarray(W1, dtype=np.float32).T  # [C, M]

    order = np.argsort(labels, kind="stable")
    slab = labels[order]

    band_sizes = _band_class_sizes()
    band_c0 = np.cumsum([0] + band_sizes)

    in_maps = []
    perms = []
    for k in range(NCORES):
        dp = np.zeros((SLOTS, D + 2), np.float32)
        w1g = np.zeros((SLOTS, M), np.float32)
        betam = np.zeros(SLOTS, np.float32)
        maskm = np.zeros(SLOTS, np.float32)
        lci = np.zeros(SLOTS, np.float32)
        perm = np.full(SLOTS, -1, np.int64)
        for b in range(NB):
            glo = k * CPC + band_c0[b]
            ghi = k * CPC + band_c0[b + 1]
            lo = np.searchsorted(slab, glo, side="left")
            hi = np.searchsorted(slab, ghi, side="left")
            n = hi - lo
            if n > R:
                raise OverflowError("band overflow")
            rows = order[lo:hi]
            s0 = b * R
            dp[s0:s0 + n, :D] = data[rows]
            dp[s0:s0 + n, D] = 1.0
            w1g[s0:s0 + n] = W1T[labels[rows]]
            betam[s0:s0 + n] = beta[rows]
            maskm[s0:s0 + n] = 1.0
            lci[s0:s0 + n] = (labels[rows] - k * CPC).astype(np.float32)
            lci[s0 + n:s0 + R] = float(band_c0[b])
            perm[s0:s0 + n] = rows
        # pack small tensors into [128, SMALLW]: slot s = rc*128 + p
        small = np.zeros((128, SMALLW), np.float32)
        F = NRC * M
        small[:, 0:F] = w1g.reshape(NRC, 128, M).transpose(1, 0, 2).reshape(128, F)
        small[:, F:F + NRC] = betam.reshape(NRC, 128).T
        small[:, F + NRC:F + 2 * NRC] = maskm.reshape(NRC, 128).T
        small[:, F + 2 * NRC:F + 3 * NRC] = lci.reshape(NRC, 128).T
        in_maps.append({"data_pad": dp, "small_in": small})
        perms.append(perm)
    return in_maps, perms


_PROGRAM_CACHE = {}


def kernel(data, source_labels, beta, centers, W1, b1, W2, b2,
           memory, memory_weights):
    data = np.asarray(data)
    source_labels = np.asarray(source_labels)
    beta = np.asarray(beta)
    centers = np.asarray(centers, dtype=np.float32)
    W1 = np.asarray(W1, dtype=np.float32)
    b1 = np.asarray(b1, dtype=np.float32)
    W2 = np.asarray(W2, dtype=np.float32)
    b2 = np.asarray(b2, dtype=np.float32)
    memory = np.asarray(memory, dtype=np.float32)
    memory_weights = np.asarray(memory_weights, dtype=np.float32)

    try:
        in_maps, perms = _prep_inputs(data, source_labels, beta, W1)
    except OverflowError:
        return _numpy_fallback(data, source_labels, beta, centers, W1, b1,
                               W2, b2, memory, memory_weights)

    consts = np.zeros(64, np.float32)
    consts[0:36] = W2.reshape(-1)
    consts[36:42] = b1
    consts[42:48] = b2
    cls_row = np.arange(CPC, dtype=np.float32)

    ident = np.eye(128, dtype=np.float32)
    for k in range(NCORES):
        in_maps[k]["ident_in"] = ident
        in_maps[k]["cls_row"] = cls_row
        in_maps[k]["consts_in"] = consts
        in_maps[k]["centers_sh"] = np.ascontiguousarray(
            centers[k * CPC * M:(k + 1) * CPC * M]
        )

    if "nc" not in _PROGRAM_CACHE:
        _PROGRAM_CACHE["nc"] = build_program()
    nc = _PROGRAM_CACHE["nc"]

    from concourse.bass_utils import run_bass_kernel_spmd
    res = run_bass_kernel_spmd(nc, in_maps, list(range(NCORES)))
    results = res.results

    return _assemble(results, perms, memory, memory_weights)


def _assemble(results, perms, memory, memory_weights):
    band_sizes = _band_class_sizes()
    band_c0 = np.cumsum([0] + band_sizes)

    loss_sum = np.float32(0.0)
    sum_v = np.zeros((BS, M), np.float32)
    mem = np.empty((C * M, D), np.float32)
    memw = np.empty(C * M, np.float32)
    for k in range(NCORES):
        r = results[k]
        loss_sum += r["loss_out"].reshape(-1)[0]
        # de-tile sumv [128, NRC*M] -> [SLOTS, M]
        sv = (r["sumv_out"].reshape(128, NRC, M)
              .transpose(1, 0, 2).reshape(SLOTS, M))
        perm = perms[k]
        valid = perm >= 0
        sum_v[perm[valid]] = sv[valid]
        mem[k * CPC * M:(k + 1) * CPC * M] = r["mem_out"].reshape(CPC * M, D)
        # de-tile memw [126, NCHUNK]
        mwt = r["memw_out"].reshape(126, NCHUNK)
        base = k * CPC * M
        gch = 0
        for b in range(NB):
            nq = 6 * band_sizes[b]
            chunks = _chunk_sizes(nq)
            qoff = 0
            for j, csz in enumerate(chunks):
                rb = base + band_c0[b] * 6 + qoff
                memw[rb:rb + csz] = mwt[:csz, gch + j]
                qoff += csz
            gch += len(chunks)

    loss = np.float32(loss_sum / (BS * D))
    new_memory = mem
    new_memory_w = memw.reshape(C * M, 1)
    if memory.any():
        new_memory = new_memory + memory.reshape(C * M, D)
    if memory_weights.any():
        new_memory_w = new_memory_w + memory_weights.reshape(C * M, 1)
    return loss, sum_v, new_memory, new_memory_w


# ---------------------- numpy fallback (safety net) --------------------------

def _numpy_fallback(data, source_labels, beta, centers, W1, b1, W2, b2,
                    memory, memory_weights):
    labels = np.asarray(source_labels)
    h = np.maximum(W1.T[labels] + b1, 0.0)
    out = 1.0 / (1.0 + np.exp(-(h @ W2.T + b2))) + EPS
    cs = np.cumsum(out, axis=1)
    val = (beta[:, None] - cs) ** 2
    w = np.exp(-np.sqrt(val + 1e-10))
    nw = w / (w.sum(axis=1, keepdims=True) + EPS + 1e-10)
    centers3 = centers.reshape(C, M, D)
    cm = np.einsum("bmd,bm->bd", centers3[labels], nw)
    loss = np.float32(np.mean((data - cm) ** 2))
    feat = data[:, None, :] * nw[:, :, None]
    new_mem = memory.reshape(C, M, D).copy()
    np.add.at(new_mem, labels, feat)
    new_mw = memory_weights.reshape(C, M).copy()
    np.add.at(new_mw, labels, nw)
    return (loss, cs.astype(np.float32),
            new_mem.reshape(C * M, D).astype(np.float32),
            new_mw.reshape(C * M, 1).astype(np.float32))


# revision 19
# speedup vs baseline: 2.4532x; 1.0749x over previous
"""Trainium2 Bass kernel for nn_MicroCommunity (scatter_memory).

Strategy: class-sharded across 8 NeuronCores.
  - Classes 0..9999 are split into 8 contiguous shards of 1250 classes.
  - Host sorts batch rows by label and routes each row to the core owning
    its class; within a core, rows are packed into 12 fixed class-bands
    (104/105 classes each), each padded to R=256 rows (max real occupancy
    for the fixed seed is ~209).
  - Each core computes the LSM weights (relu/sigmoid/cumsum/exp chain) for
    its rows, then per band builds a one-hot "T2" matrix
    T2[(c,m), b] = (label_b == c) * norm_w[b, m] and uses PE matmuls:
       memory_band   = T2 @ [data | mask]    (scatter-add + memory_weights)
       center_matrix = T2.T @ centers_band   (gather)
    Loss partials are reduced on-device; host sums 8 scalars.
  - The memory/memory_weights outputs are disjoint across cores (no
    all-reduce needed); host concatenates shards.

kernel() accepts FULL unsharded inputs and returns the FULL outputs
(loss, sum_v, new_memory, new_memory_w) exactly like the reference.
"""

import numpy as np

# ---------------- problem constants (hardcoded per contract) ----------------
BS = 16384
C = 10000
M = 6
D = 256
EPS = 1e-4
NCORES = 8
CPC = C // NCORES          # classes per core = 1250
NB = 12                    # class-bands per core
R = 256                    # padded rows per band
SLOTS = NB * R             # 3072 row slots per core
NRC = SLOTS // 128         # 24 row-chunks of 128 partitions
DENOM_C = EPS + 1e-10      # norm_w denominator epsilon
SMALLW = NRC * M + 3 * NRC  # packed small input width: 144 + 72 = 216
NCHUNK = 60                 # total (band, chunk) pairs per core
import os as _os
PE_DT = _os.environ.get("BASS_PE_DT", "f16")  # "f16" or "f32r"


def _band_class_sizes():
    base = CPC // NB
    rem = CPC - base * NB
    return [base + (1 if b < rem else 0) for b in range(NB)]  # [105,105,104*10]


def _chunk_sizes(nq):
    out = []
    while nq > 0:
        c = min(126, nq)
        out.append(c)
        nq -= c
    return out


# ---------------------------- program builder -------------------------------

def build_program():
    import concourse.bass as bass
    import concourse.bacc as bacc
    import concourse.mybir as mybir
    import concourse.tile as tile

    f32 = mybir.dt.float32
    f32r = mybir.dt.float32r
    pedt = mybir.dt.float16 if PE_DT == "f16" else f32r
    Alu = mybir.AluOpType
    Act = mybir.ActivationFunctionType

    nc = bacc.Bacc("TRN2", target_bir_lowering=False)

    # ------------- I/O -------------
    data_pad = nc.dram_tensor("data_pad", [SLOTS, D + 2], pedt, kind="ExternalInput")
    small_in = nc.dram_tensor("small_in", [128, SMALLW], f32, kind="ExternalInput")
    cls_row = nc.dram_tensor("cls_row", [CPC], f32, kind="ExternalInput")
    consts_in = nc.dram_tensor("consts_in", [64], f32, kind="ExternalInput")
    centers_sh = nc.dram_tensor("centers_sh", [CPC * M, D], pedt, kind="ExternalInput")
    ident_in = nc.dram_tensor("ident_in", [128, 128], pedt, kind="ExternalInput")

    mem_out = nc.dram_tensor("mem_out", [CPC * M, D], f32, kind="ExternalOutput")
    memw_out = nc.dram_tensor("memw_out", [126, NCHUNK], f32, kind="ExternalOutput")
    sumv_out = nc.dram_tensor("sumv_out", [128, NRC * M], f32, kind="ExternalOutput")
    loss_out = nc.dram_tensor("loss_out", [1, 1], f32, kind="ExternalOutput")

    band_sizes = _band_class_sizes()
    band_c0 = np.cumsum([0] + band_sizes).tolist()

    def apx(ap, dims, extra=0):
        return bass.AP(tensor=ap.tensor, offset=ap.offset + extra, ap=dims)

    with tile.TileContext(nc) as tc:
        with (
            tc.tile_pool(name="singles", bufs=1) as singles,
            tc.tile_pool(name="dpool", bufs=3) as dpool,
            tc.tile_pool(name="cpool", bufs=3) as cpool,
            tc.tile_pool(name="t2pool", bufs=8) as t2pool,
            tc.tile_pool(name="lspool", bufs=3) as lspool,
            tc.tile_pool(name="ohpool", bufs=3) as ohpool,
            tc.tile_pool(name="dfpool", bufs=3) as dfpool,
            tc.tile_pool(name="trpool", bufs=2) as trpool,
            tc.tile_pool(name="stpool", bufs=3) as stpool,
            tc.tile_pool(name="pT", bufs=2, space="PSUM") as pT,
            tc.tile_pool(name="pSC", bufs=3, space="PSUM") as pSC,
            tc.tile_pool(name="pCM", bufs=2, space="PSUM") as pCM,
            tc.tile_pool(name="pL", bufs=1, space="PSUM") as pL,
        ):
            # ---------------- resident tiles + loads ----------------
            ident = singles.tile([128, 128], pedt, tag="ident")
            nc.sync.dma_start(out=ident[:], in_=ident_in[:])

            clsrep = singles.tile([128, CPC], f32, tag="clsrep")
            nc.sync.dma_start(
                out=clsrep[:], in_=apx(cls_row[:], [[0, 128], [1, CPC]])
            )
            consts = singles.tile([128, 64], f32, tag="consts")
            nc.sync.dma_start(
                out=consts[:], in_=apx(consts_in[:], [[0, 128], [1, 64]])
            )

            small_t = singles.tile([128, SMALLW], f32, tag="small")
            nc.sync.dma_start(out=small_t[:], in_=small_in[:])
            st = small_t[:]
            SP = SMALLW  # partition stride of small tile
            F = NRC * M  # 144

            def w1g_v():
                return apx(st, [[SP, 128], [M, NRC], [1, M]], extra=0)

            def beta_v3():
                return apx(st, [[SP, 128], [1, NRC], [0, M]], extra=F)

            def mask_v3():
                return apx(st, [[SP, 128], [1, NRC], [0, M]], extra=F + NRC)

            def lci_col(g):
                return apx(st, [[SP, 128], [1, 1]], extra=F + 2 * NRC + g)

            # ---------------- phase A: norm_w / sum_v ----------------
            h_t = singles.tile([128, NRC, M], f32, tag="h")
            z_t = singles.tile([128, NRC, M], f32, tag="z")
            tmp_t = singles.tile([128, NRC, M], f32, tag="tmpa")
            s_t = singles.tile([128, NRC, M], f32, tag="s")
            cs_t = singles.tile([128, NRC, M], f32, tag="cs")
            w_t = singles.tile([128, NRC, M], f32, tag="w")
            nw_t = singles.tile([128, NRC, M], f32, tag="nw")
            sw_t = singles.tile([128, NRC], f32, tag="sw")
            rw_t = singles.tile([128, NRC], f32, tag="rw")
            losspart = singles.tile([128, NRC], f32, tag="losspart")

            cst = consts[:]
            PSTRIDE = 64

            c_sqrt_bias = singles.tile([128, 1], f32, tag="csqrtb")
            nc.vector.memset(c_sqrt_bias[:], 1e-10)

            def cbc(off, dims):
                return apx(cst, dims, extra=off)

            # h = relu(w1g + b1)   (b1 at consts[36:42])
            nc.vector.tensor_tensor(
                out=h_t[:], in0=w1g_v(),
                in1=cbc(36, [[PSTRIDE, 128], [0, NRC], [1, M]]),
                op=Alu.add,
            )
            nc.vector.tensor_scalar_max(out=h_t[:], in0=h_t[:], scalar1=0.0)

            # z = h @ W2.T  (W2 row-major at consts[0:36])
            ht = h_t[:]
            HS = NRC * M
            for k in range(M):
                dst = z_t[:] if k == 0 else tmp_t[:]
                nc.vector.tensor_tensor(
                    out=dst,
                    in0=apx(ht, [[HS, 128], [M, NRC], [0, M]], extra=k),
                    in1=cbc(k, [[PSTRIDE, 128], [0, NRC], [M, M]]),
                    op=Alu.mult,
                )
                if k > 0:
                    nc.vector.tensor_tensor(
                        out=z_t[:], in0=z_t[:], in1=tmp_t[:], op=Alu.add
                    )
            nc.vector.tensor_tensor(
                out=z_t[:], in0=z_t[:],
                in1=cbc(42, [[PSTRIDE, 128], [0, NRC], [1, M]]),
                op=Alu.add,
            )
            # s = sigmoid(z) + EPS
            nc.scalar.activation(out=s_t[:], in_=z_t[:], func=Act.Sigmoid)
            nc.vector.tensor_scalar_add(out=s_t[:], in0=s_t[:], scalar1=EPS)
            # cs = cumsum(s, axis=-1)
            nc.vector.tensor_copy(out=cs_t[:, :, 0], in_=s_t[:, :, 0])
            for j in range(1, M):
                nc.vector.tensor_tensor(
                    out=cs_t[:, :, j], in0=cs_t[:, :, j - 1], in1=s_t[:, :, j],
                    op=Alu.add,
                )
            # sum_v output (tiled layout, host de-tiles)
            nc.sync.dma_start(out=sumv_out[:], in_=cs_t[:])
            # w = exp(-sqrt((beta-cs)^2 + 1e-10))
            nc.vector.tensor_tensor(
                out=w_t[:], in0=beta_v3(), in1=cs_t[:], op=Alu.subtract
            )
            nc.vector.tensor_tensor(out=w_t[:], in0=w_t[:], in1=w_t[:], op=Alu.mult)
            nc.scalar.activation(
                out=w_t[:], in_=w_t[:], func=Act.Sqrt, bias=c_sqrt_bias[:]
            )
            nc.scalar.activation(out=w_t[:], in_=w_t[:], func=Act.Exp, scale=-1.0)
            # sw = sum(w) + DENOM_C ; nw = (w * recip(sw)) * mask
            nc.vector.tensor_reduce(
                out=sw_t[:], in_=w_t[:], axis=mybir.AxisListType.X, op=Alu.add
            )
            nc.vector.tensor_scalar_add(out=sw_t[:], in0=sw_t[:], scalar1=DENOM_C)
            nc.vector.tensor_tensor(
                out=w_t[:], in0=w_t[:], in1=mask_v3(), op=Alu.mult
            )
            nc.vector.reciprocal(out=rw_t[:], in_=sw_t[:])
            nc.vector.tensor_tensor(
                out=nw_t[:], in0=w_t[:],
                in1=apx(rw_t[:], [[NRC, 128], [1, NRC], [0, M]]),
                op=Alu.mult,
            )

            memw_sb = singles.tile([126, NCHUNK], f32, tag="memw")
            nc.vector.memset(memw_sb[:], 0.0)

            # ---------------- phase B: per-band gather/scatter ----------------
            gchunk = 0
            for b in range(NB):
                B = band_sizes[b]
                c0 = band_c0[b]
                nq = 6 * B
                chunks = _chunk_sizes(nq)
                qoffs = np.cumsum([0] + chunks).tolist()
                NJ = len(chunks)

                # merged loads: data (1 DMA), centers (1 DMA, scalar ring)
                dt2 = dpool.tile([128, 2, D + 2], pedt, tag="dt")
                r0 = b * R
                nc.sync.dma_start(
                    out=dt2[:],
                    in_=apx(data_pad[:], [[D + 2, 128], [128 * (D + 2), 2], [1, D + 2]],
                            extra=r0 * (D + 2)),
                )
                ct2 = cpool.tile([126, NJ, D], pedt, tag="ct")
                rbase0 = c0 * M
                if chunks[-1] == 126:
                    nc.scalar.dma_start(
                        out=ct2[:],
                        in_=apx(centers_sh[:], [[D, 126], [126 * D, NJ], [1, D]],
                                extra=rbase0 * D),
                    )
                else:
                    nc.scalar.dma_start(
                        out=ct2[:, 0:NJ - 1, :],
                        in_=apx(centers_sh[:],
                                [[D, 126], [126 * D, NJ - 1], [1, D]],
                                extra=rbase0 * D),
                    )
                    cl = chunks[-1]
                    nc.scalar.dma_start(
                        out=ct2[:cl, NJ - 1, :],
                        in_=centers_sh[rbase0 + 126 * (NJ - 1):
                                       rbase0 + 126 * (NJ - 1) + cl, :],
                    )

                # build lhsT_sc [128, nq] per row-chunk
                lhsts = []
                for rc in range(2):
                    g = b * 2 + rc
                    oht = ohpool.tile([128, 105], f32, tag="oht")
                    nc.vector.tensor_tensor(
                        out=oht[:, :B],
                        in0=clsrep[:, c0:c0 + B],
                        in1=lci_col(g).to_broadcast([128, B]),
                        op=Alu.is_equal,
                    )
                    lh = lspool.tile([128, 6 * 105], pedt, tag="lh")
                    nc.gpsimd.tensor_tensor(
                        out=lh[:, :nq],
                        in0=apx(oht[:], [[105, 128], [1, B], [0, M]]),
                        in1=apx(nw_t[:], [[NRC * M, 128], [0, B], [1, M]],
                                extra=g * M),
                        op=Alu.mult,
                    )
                    lhsts.append(lh)

                # transposes -> T2 chunks [csz, 256]
                t2s = []
                for j in range(NJ):
                    csz = chunks[j]
                    pt = pT.tile([126, 256], pedt, tag="pT")
                    for rc in range(2):
                        nc.tensor.transpose(
                            out=pt[:csz, rc * 128:(rc + 1) * 128],
                            in_=lhsts[rc][:, qoffs[j]:qoffs[j] + csz],
                            identity=ident[:],
                        )
                    t2 = t2pool.tile([126, 256], pedt, tag="t2")
                    nc.scalar.activation(
                        out=t2[:csz, :], in_=pt[:csz, :], func=Act.Copy
                    )
                    t2s.append(t2)

                # scatter matmuls + staging + memw column
                stg2 = stpool.tile([126, NJ, D], f32, tag="stg")
                for j in range(NJ):
                    csz = chunks[j]
                    psc = pSC.tile([126, D + 2], f32, tag="pSC")
                    for rc in range(2):
                        nc.tensor.matmul(
                            out=psc[:csz, :],
                            lhsT=lhsts[rc][:, qoffs[j]:qoffs[j] + csz],
                            rhs=dt2[:, rc, :],
                            start=(rc == 0),
                            stop=(rc == 1),
                        )
                    if j % 2 == 0:
                        nc.vector.tensor_copy(
                            out=stg2[:csz, j, :], in_=psc[:csz, 0:D]
                        )
                    else:
                        nc.scalar.activation(
                            out=stg2[:csz, j, :], in_=psc[:csz, 0:D], func=Act.Copy
                        )
                    gj = gchunk + j
                    nc.vector.tensor_copy(
                        out=memw_sb[:csz, gj:gj + 1], in_=psc[:csz, D:D + 1]
                    )
                if chunks[-1] == 126:
                    nc.sync.dma_start(
                        out=apx(mem_out[:], [[D, 126], [126 * D, NJ], [1, D]],
                                extra=rbase0 * D),
                        in_=stg2[:],
                    )
                else:
                    nc.sync.dma_start(
                        out=apx(mem_out[:], [[D, 126], [126 * D, NJ - 1], [1, D]],
                                extra=rbase0 * D),
                        in_=stg2[:, 0:NJ - 1, :],
                    )
                    cl = chunks[-1]
                    nc.sync.dma_start(
                        out=mem_out[rbase0 + qoffs[NJ - 1]:
                                    rbase0 + qoffs[NJ - 1] + cl, :],
                        in_=stg2[:cl, NJ - 1, :],
                    )

                # gather matmuls + loss
                for rc in range(2):
                    g = b * 2 + rc
                    pcm = pCM.tile([128, D], f32, tag="pCM")
                    for j in range(NJ):
                        csz = chunks[j]
                        nc.tensor.matmul(
                            out=pcm[:],
                            lhsT=t2s[j][:csz, rc * 128:(rc + 1) * 128],
                            rhs=ct2[:csz, j, :],
                            start=(j == 0),
                            stop=(j == NJ - 1),
                        )
                    diff = dfpool.tile([128, D], f32, tag="diff")
                    nc.vector.tensor_tensor(
                        out=diff[:], in0=dt2[:, rc, 0:D],
                        in1=pcm[:], op=Alu.subtract,
                    )
                    trash = trpool.tile([128, D], f32, tag="trash")
                    nc.scalar.activation(
                        out=trash[:], in_=diff[:], func=Act.Square,
                        accum_out=losspart[:, g:g + 1],
                    )
                gchunk += NJ

            # memory_weights output (tiled; host de-tiles)
            nc.sync.dma_start(out=memw_out[:], in_=memw_sb[:])

            # ---------------- final loss reduction ----------------
            lp1 = singles.tile([128, 1], f32, tag="lp1")
            nc.vector.tensor_reduce(
                out=lp1[:], in_=losspart[:], axis=mybir.AxisListType.X, op=Alu.add
            )
            ones_t = singles.tile([128, 1], f32, tag="ones")
            nc.vector.memset(ones_t[:], 1.0)
            pl = pL.tile([1, 1], f32, tag="pL")
            nc.tensor.matmul(
                out=pl[:], lhsT=lp1[:], rhs=ones_t[:], start=True, stop=True
            )
            lt = singles.tile([1, 1], f32, tag="lt")
            nc.vector.tensor_copy(out=lt[:], in_=pl[:])
            nc.sync.dma_start(out=loss_out[:], in_=lt[:])

    nc.compile()
    return nc


# ------------------------------ host side -----------------------------------

def _prep_inputs(data, source_labels, beta, W1):
    """Sort rows by label, pack into per-core band-padded layouts."""
    labels = np.asarray(source_labels)
    data = np.asarray(data, dtype=np.float32)
    beta = np.asarray(beta, dtype=np.float32)
    W1T = np.asarray(W1, dtype=np.float32).T  # [C, M]

    order = np.argsort(labels, kind="stable")
    slab = labels[order]

    band_sizes = _band_class_sizes()
    band_c0 = np.cumsum([0] + band_sizes)

    in_maps = []
    perms = []
    for k in range(NCORES):
        dp = np.zeros((SLOTS, D + 2), np.float32)
        w1g = np.zeros((SLOTS, M), np.float32)
        betam = np.zeros(SLOTS, np.float32)
        maskm = np.zeros(SLOTS, np.float32)
        lci = np.zeros(SLOTS, np.float32)
        perm = np.full(SLOTS, -1, np.int64)
        for b in range(NB):
            glo = k * CPC + band_c0[b]
            ghi = k * CPC + band_c0[b + 1]
            lo = np.searchsorted(slab, glo, side="left")
            hi = np.searchsorted(slab, ghi, side="left")
            n = hi - lo
            if n > R:
                raise OverflowError("band overflow")
            rows = order[lo:hi]
            s0 = b * R
            dp[s0:s0 + n, :D] = data[rows]
            dp[s0:s0 + n, D] = 1.0
            w1g[s0:s0 + n] = W1T[labels[rows]]
            betam[s0:s0 + n] = beta[rows]
            maskm[s0:s0 + n] = 1.0
            lci[s0:s0 + n] = (labels[rows] - k * CPC).astype(np.float32)
            lci[s0 + n:s0 + R] = float(band_c0[b])
            perm[s0:s0 + n] = rows
        # pack small tensors into [128, SMALLW]: slot s = rc*128 + p
        small = np.zeros((128, SMALLW), np.float32)
        F = NRC * M
        small[:, 0:F] = w1g.reshape(NRC, 128, M).transpose(1, 0, 2).reshape(128, F)
        small[:, F:F + NRC] = betam.reshape(NRC, 128).T
        small[:, F + NRC:F + 2 * NRC] = maskm.reshape(NRC, 128).T
        small[:, F + 2 * NRC:F + 3 * NRC] = lci.reshape(NRC, 128).T
        np_pe = np.float16 if PE_DT == "f16" else np.float32
        in_maps.append({"data_pad": dp.astype(np_pe), "small_in": small})
        perms.append(perm)
    return in_maps, perms


_PROGRAM_CACHE = {}


def kernel(data, source_labels, beta, centers, W1, b1, W2, b2,
           memory, memory_weights):
    data = np.asarray(data)
    source_labels = np.asarray(source_labels)
    beta = np.asarray(beta)
    centers = np.asarray(centers, dtype=np.float32)
    W1 = np.asarray(W1, dtype=np.float32)
    b1 = np.asarray(b1, dtype=np.float32)
    W2 = np.asarray(W2, dtype=np.float32)
    b2 = np.asarray(b2, dtype=np.float32)
    memory = np.asarray(memory, dtype=np.float32)
    memory_weights = np.asarray(memory_weights, dtype=np.float32)

    try:
        in_maps, perms = _prep_inputs(data, source_labels, beta, W1)
    except OverflowError:
        return _numpy_fallback(data, source_labels, beta, centers, W1, b1,
                               W2, b2, memory, memory_weights)

    consts = np.zeros(64, np.float32)
    consts[0:36] = W2.reshape(-1)
    consts[36:42] = b1
    consts[42:48] = b2
    cls_row = np.arange(CPC, dtype=np.float32)

    np_pe = np.float16 if PE_DT == "f16" else np.float32
    ident = np.eye(128, dtype=np_pe)
    for k in range(NCORES):
        in_maps[k]["ident_in"] = ident
        in_maps[k]["cls_row"] = cls_row
        in_maps[k]["consts_in"] = consts
        in_maps[k]["centers_sh"] = np.ascontiguousarray(
            centers[k * CPC * M:(k + 1) * CPC * M]
        ).astype(np_pe)

    if "nc" not in _PROGRAM_CACHE:
        _PROGRAM_CACHE["nc"] = build_program()
    nc = _PROGRAM_CACHE["nc"]

    from concourse.bass_utils import run_bass_kernel_spmd
    res = run_bass_kernel_spmd(nc, in_maps, list(range(NCORES)))
    results = res.results

    return _assemble(results, perms, memory, memory_weights)


def _assemble(results, perms, memory, memory_weights):
    band_sizes = _band_class_sizes()
    band_c0 = np.cumsum([0] + band_sizes)

    loss_sum = np.float32(0.0)
    sum_v = np.zeros((BS, M), np.float32)
    mem = np.empty((C * M, D), np.float32)
    memw = np.empty(C * M, np.float32)
    for k in range(NCORES):
        r = results[k]
        loss_sum += r["loss_out"].reshape(-1)[0]
        # de-tile sumv [128, NRC*M] -> [SLOTS, M]
        sv = (r["sumv_out"].reshape(128, NRC, M)
              .transpose(1, 0, 2).reshape(SLOTS, M))
        perm = perms[k]
        valid = perm >= 0
        sum_v[perm[valid]] = sv[valid]
        mem[k * CPC * M:(k + 1) * CPC * M] = r["mem_out"].reshape(CPC * M, D)
        # de-tile memw [126, NCHUNK]
        mwt = r["memw_out"].reshape(126, NCHUNK)
        base = k * CPC * M
        gch = 0
        for b in range(NB):
            nq = 6 * band_sizes[b]
            chunks = _chunk_sizes(nq)
            qoff = 0
            for j, csz in enumerate(chunks):
                rb = base + band_c0[b] * 6 + qoff
                memw[rb:rb + csz] = mwt[:csz, gch + j]
                qoff += csz
            gch += len(chunks)

    loss = np.float32(loss_sum / (BS * D))
    new_memory = mem
    new_memory_w = memw.reshape(C * M, 1)
    if memory.any():
        new_memory = new_memory + memory.reshape(C * M, D)
    if memory_weights.any():
        new_memory_w = new_memory_w + memory_weights.reshape(C * M, 1)
    return loss, sum_v, new_memory, new_memory_w


# ---------------------- numpy fallback (safety net) --------------------------

def _numpy_fallback(data, source_labels, beta, centers, W1, b1, W2, b2,
                    memory, memory_weights):
    labels = np.asarray(source_labels)
    h = np.maximum(W1.T[labels] + b1, 0.0)
    out = 1.0 / (1.0 + np.exp(-(h @ W2.T + b2))) + EPS
    cs = np.cumsum(out, axis=1)
    val = (beta[:, None] - cs) ** 2
    w = np.exp(-np.sqrt(val + 1e-10))
    nw = w / (w.sum(axis=1, keepdims=True) + EPS + 1e-10)
    centers3 = centers.reshape(C, M, D)
    cm = np.einsum("bmd,bm->bd", centers3[labels], nw)
    loss = np.float32(np.mean((data - cm) ** 2))
    feat = data[:, None, :] * nw[:, :, None]
    new_mem = memory.reshape(C, M, D).copy()
    np.add.at(new_mem, labels, feat)
    new_mw = memory_weights.reshape(C, M).copy()
    np.add.at(new_mw, labels, nw)
    return (loss, cs.astype(np.float32),
            new_mem.reshape(C * M, D).astype(np.float32),
            new_mw.reshape(C * M, 1).astype(np.float32))


# revision 23
# speedup vs baseline: 2.5334x; 1.0327x over previous
"""Trainium2 Bass kernel for nn_MicroCommunity (scatter_memory).

Strategy: class-sharded across 8 NeuronCores.
  - Classes 0..9999 are split into 8 contiguous shards of 1250 classes.
  - Host sorts batch rows by label and routes each row to the core owning
    its class; within a core, rows are packed into 12 fixed class-bands
    (104/105 classes each), each padded to R=256 rows (max real occupancy
    for the fixed seed is ~209).
  - Each core computes the LSM weights (relu/sigmoid/cumsum/exp chain) for
    its rows, then per band builds a one-hot "T2" matrix
    T2[(c,m), b] = (label_b == c) * norm_w[b, m] and uses PE matmuls:
       memory_band   = T2 @ [data | mask]    (scatter-add + memory_weights)
       center_matrix = T2.T @ centers_band   (gather)
    Loss partials are reduced on-device; host sums 8 scalars.
  - The memory/memory_weights outputs are disjoint across cores (no
    all-reduce needed); host concatenates shards.

kernel() accepts FULL unsharded inputs and returns the FULL outputs
(loss, sum_v, new_memory, new_memory_w) exactly like the reference.
"""

import numpy as np

# ---------------- problem constants (hardcoded per contract) ----------------
BS = 16384
C = 10000
M = 6
D = 256
EPS = 1e-4
NCORES = 8
CPC = C // NCORES          # classes per core = 1250
NB = 24                    # class-bands per core
R = 128                    # padded rows per band
SLOTS = NB * R             # 3072 row slots per core
NRC = SLOTS // 128         # 24 row-chunks of 128 partitions
DENOM_C = EPS + 1e-10      # norm_w denominator epsilon
SMALLW = NRC * M + 3 * NRC  # packed small input width: 144 + 72 = 216
NCHUNK = 72                 # total (band, chunk) pairs per core
MEMW_ROWS = 106
import os as _os
PE_DT = _os.environ.get("BASS_PE_DT", "f16")  # "f16" or "f32r"


def _band_class_sizes():
    base = CPC // NB
    rem = CPC - base * NB
    return [base + (1 if b < rem else 0) for b in range(NB)]  # [105,105,104*10]


def _chunk_sizes(nq):
    assert nq % 3 == 0
    return [nq // 3] * 3


# ---------------------------- program builder -------------------------------

def build_program():
    import concourse.bass as bass
    import concourse.bacc as bacc
    import concourse.mybir as mybir
    import concourse.tile as tile

    f32 = mybir.dt.float32
    f32r = mybir.dt.float32r
    pedt = mybir.dt.float16 if PE_DT == "f16" else f32r
    Alu = mybir.AluOpType
    Act = mybir.ActivationFunctionType

    nc = bacc.Bacc("TRN2", target_bir_lowering=False)

    # ------------- I/O -------------
    data_pad = nc.dram_tensor("data_pad", [SLOTS, D + 2], pedt, kind="ExternalInput")
    small_in = nc.dram_tensor("small_in", [128, SMALLW], f32, kind="ExternalInput")
    cls_row = nc.dram_tensor("cls_row", [CPC], f32, kind="ExternalInput")
    consts_in = nc.dram_tensor("consts_in", [64], f32, kind="ExternalInput")
    centers_sh = nc.dram_tensor("centers_sh", [CPC * M, D], pedt, kind="ExternalInput")
    ident_in = nc.dram_tensor("ident_in", [128, 128], pedt, kind="ExternalInput")

    mem_out = nc.dram_tensor("mem_out", [CPC * M, D], f32, kind="ExternalOutput")
    memw_out = nc.dram_tensor("memw_out", [MEMW_ROWS, NCHUNK], f32, kind="ExternalOutput")
    sumv_out = nc.dram_tensor("sumv_out", [128, NRC * M], f32, kind="ExternalOutput")
    loss_out = nc.dram_tensor("loss_out", [1, 1], f32, kind="ExternalOutput")

    band_sizes = _band_class_sizes()
    band_c0 = np.cumsum([0] + band_sizes).tolist()

    def apx(ap, dims, extra=0):
        return bass.AP(tensor=ap.tensor, offset=ap.offset + extra, ap=dims)

    with tile.TileContext(nc) as tc:
        with (
            tc.tile_pool(name="singles", bufs=1) as singles,
            tc.tile_pool(name="dpool", bufs=3) as dpool,
            tc.tile_pool(name="cpool", bufs=3) as cpool,
            tc.tile_pool(name="t2pool", bufs=8) as t2pool,
            tc.tile_pool(name="lspool", bufs=3) as lspool,
            tc.tile_pool(name="dfpool", bufs=3) as dfpool,
            tc.tile_pool(name="trpool", bufs=2) as trpool,
            tc.tile_pool(name="stpool", bufs=3) as stpool,
            tc.tile_pool(name="pT", bufs=2, space="PSUM") as pT,
            tc.tile_pool(name="pSC", bufs=3, space="PSUM") as pSC,
            tc.tile_pool(name="pCM", bufs=2, space="PSUM") as pCM,
            tc.tile_pool(name="pL", bufs=1, space="PSUM") as pL,
        ):
            # ---------------- resident tiles + loads ----------------
            ident = singles.tile([128, 128], pedt, tag="ident")
            nc.sync.dma_start(out=ident[:], in_=ident_in[:])

            clsrep = singles.tile([128, CPC], f32, tag="clsrep")
            nc.sync.dma_start(
                out=clsrep[:], in_=apx(cls_row[:], [[0, 128], [1, CPC]])
            )
            consts = singles.tile([128, 64], f32, tag="consts")
            nc.sync.dma_start(
                out=consts[:], in_=apx(consts_in[:], [[0, 128], [1, 64]])
            )

            small_t = singles.tile([128, SMALLW], f32, tag="small")
            nc.sync.dma_start(out=small_t[:], in_=small_in[:])
            st = small_t[:]
            SP = SMALLW  # partition stride of small tile
            F = NRC * M  # 144

            def w1g_v():
                return apx(st, [[SP, 128], [M, NRC], [1, M]], extra=0)

            def beta_v3():
                return apx(st, [[SP, 128], [1, NRC], [0, M]], extra=F)

            def mask_v3():
                return apx(st, [[SP, 128], [1, NRC], [0, M]], extra=F + NRC)

            def lci_col(g):
                return apx(st, [[SP, 128], [1, 1]], extra=F + 2 * NRC + g)

            # ---------------- phase A: norm_w / sum_v ----------------
            h_t = singles.tile([128, NRC, M], f32, tag="h")
            z_t = singles.tile([128, NRC, M], f32, tag="z")
            tmp_t = singles.tile([128, NRC, M], f32, tag="tmpa")
            s_t = singles.tile([128, NRC, M], f32, tag="s")
            cs_t = singles.tile([128, NRC, M], f32, tag="cs")
            w_t = singles.tile([128, NRC, M], f32, tag="w")
            nw_t = singles.tile([128, NRC, M], f32, tag="nw")
            sw_t = singles.tile([128, NRC], f32, tag="sw")
            rw_t = singles.tile([128, NRC], f32, tag="rw")
            losspart = singles.tile([128, NRC], f32, tag="losspart")

            cst = consts[:]
            PSTRIDE = 64

            c_sqrt_bias = singles.tile([128, 1], f32, tag="csqrtb")
            nc.vector.memset(c_sqrt_bias[:], 1e-10)

            def cbc(off, dims):
                return apx(cst, dims, extra=off)

            # h = relu(w1g + b1)   (b1 at consts[36:42])
            nc.vector.tensor_tensor(
                out=h_t[:], in0=w1g_v(),
                in1=cbc(36, [[PSTRIDE, 128], [0, NRC], [1, M]]),
                op=Alu.add,
            )
            nc.vector.tensor_scalar_max(out=h_t[:], in0=h_t[:], scalar1=0.0)

            # z = h @ W2.T  (W2 row-major at consts[0:36])
            ht = h_t[:]
            HS = NRC * M
            for k in range(M):
                dst = z_t[:] if k == 0 else tmp_t[:]
                nc.vector.tensor_tensor(
                    out=dst,
                    in0=apx(ht, [[HS, 128], [M, NRC], [0, M]], extra=k),
                    in1=cbc(k, [[PSTRIDE, 128], [0, NRC], [M, M]]),
                    op=Alu.mult,
                )
                if k > 0:
                    nc.vector.tensor_tensor(
                        out=z_t[:], in0=z_t[:], in1=tmp_t[:], op=Alu.add
                    )
            nc.vector.tensor_tensor(
                out=z_t[:], in0=z_t[:],
                in1=cbc(42, [[PSTRIDE, 128], [0, NRC], [1, M]]),
                op=Alu.add,
            )
            # s = sigmoid(z) + EPS
            nc.scalar.activation(out=s_t[:], in_=z_t[:], func=Act.Sigmoid)
            nc.vector.tensor_scalar_add(out=s_t[:], in0=s_t[:], scalar1=EPS)
            # cs = cumsum(s, axis=-1)
            nc.vector.tensor_copy(out=cs_t[:, :, 0], in_=s_t[:, :, 0])
            for j in range(1, M):
                nc.vector.tensor_tensor(
                    out=cs_t[:, :, j], in0=cs_t[:, :, j - 1], in1=s_t[:, :, j],
                    op=Alu.add,
                )
            # sum_v output (tiled layout, host de-tiles)
            nc.sync.dma_start(out=sumv_out[:], in_=cs_t[:])
            # w = exp(-sqrt((beta-cs)^2 + 1e-10))
            nc.vector.tensor_tensor(
                out=w_t[:], in0=beta_v3(), in1=cs_t[:], op=Alu.subtract
            )
            nc.vector.tensor_tensor(out=w_t[:], in0=w_t[:], in1=w_t[:], op=Alu.mult)
            nc.scalar.activation(
                out=w_t[:], in_=w_t[:], func=Act.Sqrt, bias=c_sqrt_bias[:]
            )
            nc.scalar.activation(out=w_t[:], in_=w_t[:], func=Act.Exp, scale=-1.0)
            # sw = sum(w) + DENOM_C ; nw = (w * recip(sw)) * mask
            nc.vector.tensor_reduce(
                out=sw_t[:], in_=w_t[:], axis=mybir.AxisListType.X, op=Alu.add
            )
            nc.vector.tensor_scalar_add(out=sw_t[:], in0=sw_t[:], scalar1=DENOM_C)
            nc.vector.tensor_tensor(
                out=w_t[:], in0=w_t[:], in1=mask_v3(), op=Alu.mult
            )
            nc.vector.reciprocal(out=rw_t[:], in_=sw_t[:])
            nc.vector.tensor_tensor(
                out=nw_t[:], in0=w_t[:],
                in1=apx(rw_t[:], [[NRC, 128], [1, NRC], [0, M]]),
                op=Alu.mult,
            )

            memw_sb = singles.tile([MEMW_ROWS, NCHUNK], f32, tag="memw")
            nc.vector.memset(memw_sb[:], 0.0)

            # ---------------- phase B: per-band gather/scatter ----------------
            gchunk = 0
            for b in range(NB):
                B = band_sizes[b]
                c0 = band_c0[b]
                nq = 6 * B
                chunks = _chunk_sizes(nq)
                csz = chunks[0]
                qoffs = np.cumsum([0] + chunks).tolist()
                NJ = len(chunks)
                rbase0 = c0 * M
                pair = b // 2
                half = b % 2

                # paired loads: one data DMA + one centers DMA per 2 bands
                if half == 0:
                    dt2 = dpool.tile([128, 2, D + 2], pedt, tag="dt")
                    r0 = b * R
                    nc.sync.dma_start(
                        out=dt2[:],
                        in_=apx(data_pad[:],
                                [[D + 2, 128], [128 * (D + 2), 2], [1, D + 2]],
                                extra=r0 * (D + 2)),
                    )
                    ct2 = cpool.tile([MEMW_ROWS, 2 * NJ, D], pedt, tag="ct")
                    nc.scalar.dma_start(
                        out=ct2[:csz, :, :],
                        in_=apx(centers_sh[:],
                                [[D, csz], [csz * D, 2 * NJ], [1, D]],
                                extra=rbase0 * D),
                    )
                    stg2 = stpool.tile([MEMW_ROWS, 2 * NJ, D], f32, tag="stg")
                    pair_rbase = rbase0
                    pair_state = (dt2, ct2, stg2, pair_rbase)
                else:
                    dt2, ct2, stg2, pair_rbase = pair_state

                dts = dt2[:, half, :]

                # fused one-hot * nw build: lh[p, q] = (cls(q)==lci_p) * nw[p, m(q)]
                lh = lspool.tile([128, 6 * 53], pedt, tag="lh")
                nc.vector.scalar_tensor_tensor(
                    out=lh[:, :nq],
                    in0=apx(clsrep[:], [[CPC, 128], [1, B], [0, M]], extra=c0),
                    scalar=lci_col(b),
                    in1=apx(nw_t[:], [[NRC * M, 128], [0, B], [1, M]],
                            extra=b * M),
                    op0=Alu.is_equal,
                    op1=Alu.mult,
                )

                # transposes -> T2 chunks [csz, 128]
                t2s = []
                for j in range(NJ):
                    pt = pT.tile([MEMW_ROWS, 128], pedt, tag="pT")
                    nc.tensor.transpose(
                        out=pt[:csz, :],
                        in_=lh[:, qoffs[j]:qoffs[j] + csz],
                        identity=ident[:],
                    )
                    t2 = t2pool.tile([MEMW_ROWS, 128], pedt, tag="t2")
                    nc.scalar.activation(
                        out=t2[:csz, :], in_=pt[:csz, :], func=Act.Copy
                    )
                    t2s.append(t2)

                # scatter matmuls + staging + memw column
                for j in range(NJ):
                    psc = pSC.tile([MEMW_ROWS, D + 2], f32, tag="pSC")
                    nc.tensor.matmul(
                        out=psc[:csz, :],
                        lhsT=lh[:, qoffs[j]:qoffs[j] + csz],
                        rhs=dts,
                        start=True,
                        stop=True,
                    )
                    jj = half * NJ + j
                    if j % 2 == 0:
                        nc.vector.tensor_copy(
                            out=stg2[:csz, jj, :], in_=psc[:csz, 0:D]
                        )
                    else:
                        nc.scalar.activation(
                            out=stg2[:csz, jj, :], in_=psc[:csz, 0:D], func=Act.Copy
                        )
                    gj = gchunk + j
                    nc.vector.tensor_copy(
                        out=memw_sb[:csz, gj:gj + 1], in_=psc[:csz, D:D + 1]
                    )
                if half == 1:
                    nc.sync.dma_start(
                        out=apx(mem_out[:], [[D, csz], [csz * D, 2 * NJ], [1, D]],
                                extra=pair_rbase * D),
                        in_=stg2[:csz, :, :],
                    )

                # gather matmuls + loss
                pcm = pCM.tile([128, D], f32, tag="pCM")
                for j in range(NJ):
                    nc.tensor.matmul(
                        out=pcm[:],
                        lhsT=t2s[j][:csz, :],
                        rhs=ct2[:csz, half * NJ + j, :],
                        start=(j == 0),
                        stop=(j == NJ - 1),
                    )
                diff = dfpool.tile([128, D], f32, tag="diff")
                nc.vector.tensor_tensor(
                    out=diff[:], in0=dts[:, 0:D],
                    in1=pcm[:], op=Alu.subtract,
                )
                trash = trpool.tile([128, D], f32, tag="trash")
                nc.scalar.activation(
                    out=trash[:], in_=diff[:], func=Act.Square,
                    accum_out=losspart[:, b:b + 1],
                )
                gchunk += NJ

            # memory_weights output (tiled; host de-tiles)
            nc.sync.dma_start(out=memw_out[:], in_=memw_sb[:])

            # ---------------- final loss reduction ----------------
            lp1 = singles.tile([128, 1], f32, tag="lp1")
            nc.vector.tensor_reduce(
                out=lp1[:], in_=losspart[:], axis=mybir.AxisListType.X, op=Alu.add
            )
            ones_t = singles.tile([128, 1], f32, tag="ones")
            nc.vector.memset(ones_t[:], 1.0)
            pl = pL.tile([1, 1], f32, tag="pL")
            nc.tensor.matmul(
                out=pl[:], lhsT=lp1[:], rhs=ones_t[:], start=True, stop=True
            )
            lt = singles.tile([1, 1], f32, tag="lt")
            nc.vector.tensor_copy(out=lt[:], in_=pl[:])
            nc.sync.dma_start(out=loss_out[:], in_=lt[:])

    nc.compile()
    return nc


# ------------------------------ host side -----------------------------------

def _prep_inputs(data, source_labels, beta, W1):
    """Sort rows by label, pack into per-core band-padded layouts."""
    labels = np.asarray(source_labels)
    data = np.asarray(data, dtype=np.float32)
    beta = np.asarray(beta, dtype=np.float32)
    W1T = np.asarray(W1, dtype=np.float32).T  # [C, M]

    order = np.argsort(labels, kind="stable")
    slab = labels[order]

    band_sizes = _band_class_sizes()
    band_c0 = np.cumsum([0] + band_sizes)

    in_maps = []
    perms = []
    for k in range(NCORES):
        dp = np.zeros((SLOTS, D + 2), np.float32)
        w1g = np.zeros((SLOTS, M), np.float32)
        betam = np.zeros(SLOTS, np.float32)
        maskm = np.zeros(SLOTS, np.float32)
        lci = np.zeros(SLOTS, np.float32)
        perm = np.full(SLOTS, -1, np.int64)
        for b in range(NB):
            glo = k * CPC + band_c0[b]
            ghi = k * CPC + band_c0[b + 1]
            lo = np.searchsorted(slab, glo, side="left")
            hi = np.searchsorted(slab, ghi, side="left")
            n = hi - lo
            if n > R:
                raise OverflowError("band overflow")
            rows = order[lo:hi]
            s0 = b * R
            dp[s0:s0 + n, :D] = data[rows]
            dp[s0:s0 + n, D] = 1.0
            w1g[s0:s0 + n] = W1T[labels[rows]]
            betam[s0:s0 + n] = beta[rows]
            maskm[s0:s0 + n] = 1.0
            lci[s0:s0 + n] = (labels[rows] - k * CPC).astype(np.float32)
            lci[s0 + n:s0 + R] = float(band_c0[b])
            perm[s0:s0 + n] = rows
        # pack small tensors into [128, SMALLW]: slot s = rc*128 + p
        small = np.zeros((128, SMALLW), np.float32)
        F = NRC * M
        small[:, 0:F] = w1g.reshape(NRC, 128, M).transpose(1, 0, 2).reshape(128, F)
        small[:, F:F + NRC] = betam.reshape(NRC, 128).T
        small[:, F + NRC:F + 2 * NRC] = maskm.reshape(NRC, 128).T
        small[:, F + 2 * NRC:F + 3 * NRC] = lci.reshape(NRC, 128).T
        np_pe = np.float16 if PE_DT == "f16" else np.float32
        in_maps.append({"data_pad": dp.astype(np_pe), "small_in": small})
        perms.append(perm)
    return in_maps, perms


_PROGRAM_CACHE = {}


def kernel(data, source_labels, beta, centers, W1, b1, W2, b2,
           memory, memory_weights):
    data = np.asarray(data)
    source_labels = np.asarray(source_labels)
    beta = np.asarray(beta)
    centers = np.asarray(centers, dtype=np.float32)
    W1 = np.asarray(W1, dtype=np.float32)
    b1 = np.asarray(b1, dtype=np.float32)
    W2 = np.asarray(W2, dtype=np.float32)
    b2 = np.asarray(b2, dtype=np.float32)
    memory = np.asarray(memory, dtype=np.float32)
    memory_weights = np.asarray(memory_weights, dtype=np.float32)

    try:
        in_maps, perms = _prep_inputs(data, source_labels, beta, W1)
    except OverflowError:
        return _numpy_fallback(data, source_labels, beta, centers, W1, b1,
                               W2, b2, memory, memory_weights)

    consts = np.zeros(64, np.float32)
    consts[0:36] = W2.reshape(-1)
    consts[36:42] = b1
    consts[42:48] = b2
    cls_row = np.arange(CPC, dtype=np.float32)

    np_pe = np.float16 if PE_DT == "f16" else np.float32
    ident = np.eye(128, dtype=np_pe)
    for k in range(NCORES):
        in_maps[k]["ident_in"] = ident
        in_maps[k]["cls_row"] = cls_row
        in_maps[k]["consts_in"] = consts
        in_maps[k]["centers_sh"] = np.ascontiguousarray(
            centers[k * CPC * M:(k + 1) * CPC * M]
        ).astype(np_pe)

    if "nc" not in _PROGRAM_CACHE:
        _PROGRAM_CACHE["nc"] = build_program()
    nc = _PROGRAM_CACHE["nc"]

    from concourse.bass_utils import run_bass_kernel_spmd
    res = run_bass_kernel_spmd(nc, in_maps, list(range(NCORES)))
    results = res.results

    return _assemble(results, perms, memory, memory_weights)


def _assemble(results, perms, memory, memory_weights):
    band_sizes = _band_class_sizes()
    band_c0 = np.cumsum([0] + band_sizes)

    loss_sum = np.float32(0.0)
    sum_v = np.zeros((BS, M), np.float32)
    mem = np.empty((C * M, D), np.float32)
    memw = np.empty(C * M, np.float32)
    for k in range(NCORES):
        r = results[k]
        loss_sum += r["loss_out"].reshape(-1)[0]
        # de-tile sumv [128, NRC*M] -> [SLOTS, M]
        sv = (r["sumv_out"].reshape(128, NRC, M)
              .transpose(1, 0, 2).reshape(SLOTS, M))
        perm = perms[k]
        valid = perm >= 0
        sum_v[perm[valid]] = sv[valid]
        mem[k * CPC * M:(k + 1) * CPC * M] = r["mem_out"].reshape(CPC * M, D)
        # de-tile memw [126, NCHUNK]
        mwt = r["memw_out"].reshape(MEMW_ROWS, NCHUNK)
        base = k * CPC * M
        gch = 0
        for b in range(NB):
            nq = 6 * band_sizes[b]
            chunks = _chunk_sizes(nq)
            qoff = 0
            for j, csz in enumerate(chunks):
                rb = base + band_c0[b] * 6 + qoff
                memw[rb:rb + csz] = mwt[:csz, gch + j]
                qoff += csz
            gch += len(chunks)

    loss = np.float32(loss_sum / (BS * D))
    new_memory = mem
    new_memory_w = memw.reshape(C * M, 1)
    if memory.any():
        new_memory = new_memory + memory.reshape(C * M, D)
    if memory_weights.any():
        new_memory_w = new_memory_w + memory_weights.reshape(C * M, 1)
    return loss, sum_v, new_memory, new_memory_w


# ---------------------- numpy fallback (safety net) --------------------------

def _numpy_fallback(data, source_labels, beta, centers, W1, b1, W2, b2,
                    memory, memory_weights):
    labels = np.asarray(source_labels)
    h = np.maximum(W1.T[labels] + b1, 0.0)
    out = 1.0 / (1.0 + np.exp(-(h @ W2.T + b2))) + EPS
    cs = np.cumsum(out, axis=1)
    val = (beta[:, None] - cs) ** 2
    w = np.exp(-np.sqrt(val + 1e-10))
    nw = w / (w.sum(axis=1, keepdims=True) + EPS + 1e-10)
    centers3 = centers.reshape(C, M, D)
    cm = np.einsum("bmd,bm->bd", centers3[labels], nw)
    loss = np.float32(np.mean((data - cm) ** 2))
    feat = data[:, None, :] * nw[:, :, None]
    new_mem = memory.reshape(C, M, D).copy()
    np.add.at(new_mem, labels, feat)
    new_mw = memory_weights.reshape(C, M).copy()
    np.add.at(new_mw, labels, nw)
    return (loss, cs.astype(np.float32),
            new_mem.reshape(C * M, D).astype(np.float32),
            new_mw.reshape(C * M, 1).astype(np.float32))
